# revision 37
# baseline (speedup 1.0000x reference)
"""Causal self-attention (B=4, T=2048, C=1024, H=16) on 8 trn2 NeuronCores.

Sharding: 4-way tensor parallel over heads x 2-way data parallel over batch.
Core i handles head group tp = i % 4 (4 heads) for batches [2*dp, 2*dp+1],
dp = i // 4.  Each core:
  - Wqkv columns for its 4 heads ([1024, 768] bf16, q-part pre-scaled 1/8)
  - Wproj rows for its heads ([256, 1024] bf16)
  - x for its 2 batches, transposed to [C, 2T] bf16 on host
Each core computes a partial projection [C, 2T] bf16; the host sums the 4
TP partials per batch pair in fp32 and un-transposes.

All PE operands are bf16 (f32r streams ~1.5-2x slower on HW and fp32
weight loads are 2x slower).  On-core dataflow:
  A) QKV^T per 512-token l-tile: 6 single-bank accumulation passes
     (q/k/v x 2 head-groups); q/k copied to SBUF bf16; v staged bf16 and
     PE-transposed (2 heads per transpose) into V-natural layout vaug
     with a trailing ones row (row sums l for free in PV).
  B) attention per (b, h, 512-wide i-tile): S^T j-tile pairs share a
     2-bank PSUM tile and a common query window (both halves start at the
     even tile's clip offset), ONE exp per pair (bf16 out), causal masking
     via two small bf16 mask multiplies on DVE, Y^T accumulated per j-tile.
     Softmax normalization: ACT copies l to SBUF, DVE fast reciprocal,
     gpsimd partition_broadcast to 64 lanes, one DVE multiply.
  C) output projection per i-tile (2-pass K=256 accumulation, bf16 copy
     out on DVE -> DMA).
  QKV work for batch b+1 and projection work are emitted as fine-grained
  FILLER between attention pairs so the tensor queue never drains.
"""

import numpy as np
from contextlib import ExitStack

import ml_dtypes

import concourse.bacc as bacc
import concourse.bass as bass
import concourse.mybir as mybir
import concourse.tile as tile
from concourse.bass_utils import run_bass_kernel_spmd

NCORES = 8
TPC = 4                # tensor-parallel cores (head groups)
DPC = 2                # data-parallel groups
C = 1024
H = 16
D = 64                 # head dim
HPC = H // TPC         # heads per core = 4
HG = HPC // 2          # head groups of 2 per core = 2
FPC = HPC * D          # features per core = 256
KC = C // 128          # contraction chunks = 8
SCALE = 1.0 / 8.0      # 1/sqrt(D)

F32 = mybir.dt.float32
BF16 = mybir.dt.bfloat16
FP8 = mybir.dt.float8e4
AF = mybir.ActivationFunctionType

_CACHE = {}
LAST_RESULT = None


def build_program(B, T):
    BL = B // DPC          # local batches = 2
    R = BL * T             # local tokens = 4096
    TJ = T // 128          # 128-wide j (key) tiles per sequence = 16
    TI = T // 512          # 512-wide i (query) tiles per sequence = 4
    SB = HPC * TJ          # vaug stripes per batch = 64
    assert T % 512 == 0

    nc = bacc.Bacc("TRN2", target_bir_lowering=False, debug=False,
                   num_devices=NCORES)
    xT = nc.dram_tensor("xT", [C, R], BF16, kind="ExternalInput").ap()
    xT8 = nc.dram_tensor("xT8", [C, R], FP8, kind="ExternalInput").ap()
    w8 = nc.dram_tensor("w8", [C, 2 * FPC], FP8,
                        kind="ExternalInput").ap()
    wv = nc.dram_tensor("wv", [C, FPC], BF16,
                        kind="ExternalInput").ap()
    wp = nc.dram_tensor("wp", [FPC, C], BF16, kind="ExternalInput").ap()
    ident = nc.dram_tensor("ident", [128, 128], BF16,
                           kind="ExternalInput").ap()
    ones64 = nc.dram_tensor("ones64", [65, 64], BF16,
                            kind="ExternalInput").ap()
    vones = nc.dram_tensor("vones", [128, BL * SB], BF16,
                           kind="ExternalInput").ap()
    # masks[:, 0:128]  = tril (keep col >= row)
    # masks[:, 128:384] = keep col-128 >= row (odd half of a pair window)
    masks = nc.dram_tensor("masks", [128, 384], BF16,
                           kind="ExternalInput").ap()
    outT = nc.dram_tensor("outT", [C, R], BF16, kind="ExternalOutput").ap()

    with tile.TileContext(nc) as tc, ExitStack() as ctx:
        const = ctx.enter_context(tc.tile_pool(name="const", bufs=1))
        big = ctx.enter_context(tc.tile_pool(name="big", bufs=1))
        xpool = ctx.enter_context(tc.tile_pool(name="xpool", bufs=32))
        xpool8 = ctx.enter_context(tc.tile_pool(name="xpool8", bufs=16))
        vspool = ctx.enter_context(tc.tile_pool(name="vspool", bufs=2))
        ptpool = ctx.enter_context(tc.tile_pool(name="ptpool", bufs=6))
        lpool = ctx.enter_context(tc.tile_pool(name="lpool", bufs=2))
        rpool = ctx.enter_context(tc.tile_pool(name="rpool", bufs=2))
        bcpool = ctx.enter_context(tc.tile_pool(name="bcpool", bufs=2))
        ystpool = ctx.enter_context(tc.tile_pool(name="ystpool", bufs=2))
        opool = ctx.enter_context(tc.tile_pool(name="opool", bufs=3))
        psA = ctx.enter_context(tc.tile_pool(name="psA", bufs=2, space="PSUM"))
        psS = ctx.enter_context(tc.tile_pool(name="psS", bufs=2, space="PSUM"))
        psY = ctx.enter_context(tc.tile_pool(name="psY", bufs=2, space="PSUM"))

        # ---- constants (ordered so phase A can start early) ----
        w8_sb = const.tile([128, KC // 2, 2, 2 * FPC], FP8)
        nc.sync.dma_start(out=w8_sb,
                          in_=w8.rearrange("(kp two p) c -> p kp two c",
                                           two=2, p=128))
        mask_sb = const.tile([128, 384], BF16)
        nc.sync.dma_start(out=mask_sb, in_=masks)
        wv_sb = const.tile([128, KC, FPC], BF16)
        nc.sync.dma_start(out=wv_sb,
                          in_=wv.rearrange("(kc p) c -> p kc c", p=128))
        identcol = const.tile([128, 128], BF16)
        nc.sync.dma_start(out=identcol, in_=ident)
        ones_sb = const.tile([65, 64], BF16)
        nc.sync.dma_start(out=ones_sb, in_=ones64)
        wp_sb = const.tile([128, 2, C], BF16)
        nc.sync.dma_start(out=wp_sb,
                          in_=wp.rearrange("(kc p) c -> p kc c", p=128))

        # per-batch persistent tiles
        qkts, yts, vaugs = [], [], []
        for b in range(BL):
            qkts.append(big.tile([128, HG, 2, T], BF16, name=f"qkt{b}",
                                 tag=f"qkt{b}"))
            yts.append(big.tile([128, 2, T], BF16, name=f"yt{b}",
                                tag=f"yt{b}"))
            v = big.tile([128, D + 1, SB], BF16, name=f"va{b}", tag=f"va{b}")
            nc.sync.dma_start(out=v[:, D, :],
                              in_=vones[:, b * SB:(b + 1) * SB])
            vaugs.append(v)

        xT3 = xT.rearrange("(kc p) r -> p kc r", p=128)
        xT83 = xT8.rearrange("(kp two p) r -> p kp two r", two=2, p=128)
        outTr = outT.rearrange("(cc p) r -> cc p r", p=128)

        # ---------- phase A unit generators (QKV projection) ----------
        xts_store = {}

        def ci_units(b, lt, ci):
            """Accumulation matmuls + 1 epilogue unit for one 128-col
            weight slice of l-tile (b, lt).  q/k slices (ci<4) run as
            fp8 DoubleRow over k-chunk pairs; v slices run bf16."""
            qkt, vaug = qkts[b], vaugs[b]
            l0 = lt * 512
            store = xts_store[(b, lt)]
            ps_box = []

            if ci < 4:
                def a_mm8(kp, ci=ci, ps_box=ps_box, store=store):
                    if not ps_box:
                        ps_box.append(
                            psA.tile([128, 512], F32, tag="a", name="psa"))
                    nc.tensor.matmul(
                        ps_box[0][:, :],
                        lhsT=w8_sb[:, kp, :, ci * 128:(ci + 1) * 128],
                        rhs=store["f8"][kp][:, :, :],
                        start=(kp == 0), stop=(kp == KC // 2 - 1),
                        perf_mode=mybir.MatmulPerfMode.DoubleRow,
                    )

                for kp in range(KC // 2):
                    yield lambda kp=kp, f=a_mm8: f(kp)

                # q stored as 32*Wq/8, k as 32*Wk: descale on copy out
                def a_copy(ci=ci, ps_box=ps_box, l0=l0, qkt=qkt):
                    qk, hg = ci // 2, ci % 2
                    nc.vector.tensor_scalar_mul(
                        qkt[:, hg, qk, l0:l0 + 512], ps_box[0][:, :],
                        1.0 / 32.0)

                yield a_copy
                return

            def a_mm(k, ci=ci, ps_box=ps_box, store=store):
                if not ps_box:
                    ps_box.append(
                        psA.tile([128, 512], F32, tag="a", name="psa"))
                nc.tensor.matmul(
                    ps_box[0][:, :],
                    lhsT=wv_sb[:, k, (ci - 4) * 128:(ci - 3) * 128],
                    rhs=store["bf"][k][:, :],
                    start=(k == 0), stop=(k == KC - 1),
                )

            for k in range(KC):
                yield lambda k=k, f=a_mm: f(k)

            if False:
                yield None
            else:
                # v head-group hg = ci-4: stage, transpose, store
                def v_stage(ci=ci, ps_box=ps_box, lt=lt, vaug=vaug):
                    hg = ci - 4
                    vstage = vspool.tile([128, 512], BF16, tag="vs",
                                         name="vstage")
                    nc.vector.tensor_copy(vstage[:, :], ps_box[0][:, :])
                    ps_t = psA.tile([128, 512], BF16, tag="a",
                                    name="pst")
                    for jb in range(4):
                        nc.tensor.matmul(
                            ps_t[:, jb * 128:(jb + 1) * 128],
                            lhsT=vstage[:, jb * 128:(jb + 1) * 128],
                            rhs=identcol[:, :],
                            is_transpose=True,
                            start=(jb == 0), stop=(jb == 3),
                            skip_group_check=True,
                        )
                    # stripes: head h=2*hg+hh at (2*hg+hh)*TJ + lt*4
                    va_r = vaug[:, 0:D, :].rearrange(
                        "p d (h t j) -> p d h t j",
                        h=HPC, t=TI, j=4)
                    nc.vector.tensor_copy(
                        va_r[:, :, 2 * hg:2 * hg + 2, lt, :],
                        ps_t.rearrange("p (jb hh d) -> p d hh jb",
                                       jb=4, hh=2))

                yield v_stage

        def v_units(b, lt):
            r0 = b * T + lt * 512
            store = xts_store[(b, lt)]

            def dma_unit(k, r0=r0, store=store):
                xt = xpool.tile([128, 512], BF16, tag="xt",
                                name=f"xtv{b}_{k}")
                nc.sync.dma_start(out=xt,
                                  in_=xT3[:, k, r0:r0 + 512])
                store["bf"].append(xt)

            for k in range(KC):
                yield lambda k=k, f=dma_unit: f(k)
            for ci in (4, 5):
                yield from ci_units(b, lt, ci)

        def a_units(b, defer_v=False):
            """Yield emission closures for batch b's QKV projection."""
            for lt in range(TI):
                r0 = b * T + lt * 512
                store = xts_store.setdefault((b, lt),
                                             {"bf": [], "f8": []})

                def dma8_unit(kp, r0=r0, store=store):
                    xt = xpool8.tile([128, 2, 512], FP8, tag="x8",
                                     name=f"x8{b}_{kp}")
                    nc.sync.dma_start(out=xt,
                                      in_=xT83[:, kp, :, r0:r0 + 512])
                    store["f8"].append(xt)

                for kp in range(KC // 2):
                    yield lambda kp=kp, f=dma8_unit: f(kp)

                for ci in range(4):
                    yield from ci_units(b, lt, ci)
                if not defer_v:
                    yield from v_units(b, lt)

        filler = []

        def pump(n):
            for _ in range(n):
                if not filler:
                    return
                filler.pop(0)()

        # ---------- phase C unit generator (output projection) ----------
        def c_units(b, i0, yt):
            w0 = b * T + i0
            for ct in range(KC):
                box = []

                def c_mm(kc, ct=ct, box=box, yt=yt, i0=i0):
                    if not box:
                        box.append(psA.tile([128, 512], F32, tag="a",
                                            name="pso"))
                    nc.tensor.matmul(
                        box[0][:, :],
                        lhsT=wp_sb[:, kc, ct * 128:(ct + 1) * 128],
                        rhs=yt[:, kc, i0:i0 + 512],
                        start=(kc == 0), stop=(kc == 1),
                    )

                for kc in range(2):
                    yield lambda kc=kc, f=c_mm: f(kc)

                def c_out(ct=ct, box=box, w0=w0):
                    ost = opool.tile([128, 512], BF16, tag="o", name="ost")
                    nc.vector.tensor_copy(ost[:, :], box[0][:, :])
                    nc.sync.dma_start(
                        out=outTr[ct, :, w0:w0 + 512],
                        in_=ost[:, :],
                    )

                yield c_out

        # ---------- phase B stream factory ----------
        def make_stream(b, it, h):
            qkt, yt, vaug = qkts[b], yts[b], vaugs[b]
            i0 = it * 512
            njt = (i0 + 512) // 128
            npair = njt // 2
            hg, hh = h // 2, h % 2
            p0 = 64 * hh
            ps_y = psY.tile([D + 1, 512], F32, tag="y", name="psy")
            pts = [None] * npair

            def s_pair(p):
                # both halves share the even tile's query window; the odd
                # strip is laid contiguously at [w, 2w) so one exp covers
                # exactly the written region
                off = max(0, 2 * p * 128 - i0)
                w = 512 - off
                straddle = (2 * p + 1) * 128 > i0
                ps_s = psS.tile([128, 1024], F32, tag="s", name="pss")
                pt = ptpool.tile([128, 1024], BF16, tag="pt", name="pt")
                for half in range(2):
                    j0 = (2 * p + half) * 128
                    nc.tensor.matmul(
                        ps_s[:, half * w:half * w + w],
                        lhsT=qkt[p0:p0 + 64, hg, 1, j0:j0 + 128],
                        rhs=qkt[p0:p0 + 64, hg, 0, i0 + off:i0 + 512],
                        start=True, stop=True,
                    )
                nc.scalar.activation(pt[:, 0:2 * w], ps_s[:, 0:2 * w],
                                     AF.Exp)
                if straddle:
                    nc.vector.tensor_tensor(
                        out=pt[:, 0:128], in0=pt[:, 0:128],
                        in1=mask_sb[:, 0:128],
                        op=mybir.AluOpType.mult)
                    nc.vector.tensor_tensor(
                        out=pt[:, w:w + 256],
                        in0=pt[:, w:w + 256],
                        in1=mask_sb[:, 128:384],
                        op=mybir.AluOpType.mult)
                pts[p] = pt

            def y_pair(p):
                pt = pts[p]
                off = max(0, 2 * p * 128 - i0)
                w = 512 - off
                for half in range(2):
                    jj = 2 * p + half
                    nc.tensor.matmul(
                        ps_y[:, off:512],
                        lhsT=vaug[:, :, h * TJ + jj],
                        rhs=pt[:, half * w:half * w + w],
                        start=(jj == 0), stop=(jj == njt - 1),
                    )

            def norm():
                # l -> SBUF f32r (DVE), PE ones-broadcast to 64 lanes,
                # copy out (ACT), reciprocal + multiply (DVE)
                lsb = lpool.tile([D + 1, 512], BF16, tag="l", name="lsb")
                with nc.allow_low_precision(reason="l to bf16 for bcast"):
                    nc.vector.tensor_copy(lsb[D:D + 1, :], ps_y[D:D + 1, :])
                ps_b = psS.tile([64, 512], F32, tag="s", name="psb")
                nc.tensor.matmul(ps_b[:, :], lhsT=ones_sb[64:65, :],
                                 rhs=lsb[D:D + 1, :],
                                 start=True, stop=True)
                bcl = rpool.tile([64, 512], F32, tag="r", name="bcl")
                nc.vector.tensor_copy(bcl[:, :], ps_b[:, :])
                bc = bcpool.tile([64, 512], F32, tag="bc", name="bc")
                nc.vector.reciprocal_approx_fast(out=bc[:, :], in_=bcl[:, :])
                if hh == 0:
                    nc.vector.tensor_mul(yt[0:64, hg, i0:i0 + 512],
                                         ps_y[0:D, :], bc[:, :])
                else:
                    yst = ystpool.tile([64, 512], BF16, tag="yst",
                                       name="yst")
                    nc.vector.tensor_mul(yst[:, :], ps_y[0:D, :], bc[:, :])
                    nc.sync.dma_start(out=yt[64:128, hg, i0:i0 + 512],
                                      in_=yst[:, :])

            return s_pair, y_pair, norm, npair

        # ---------- prologue: batch 0 q/k straight through, v deferred ----
        for u in a_units(0, defer_v=True):
            u()
        for u in v_units(0, 0):
            u()
        for lt in range(1, TI):
            filler.extend(v_units(0, lt))

        # ---------- main loop ----------
        vdef1 = []     # batch-1 v work deferred into batch 1's attention
        for b in range(BL):
            last = (b + 1 == BL)
            if not last:
                filler.extend(a_units(b + 1, defer_v=True))
                for lt in range(TI):
                    if lt < 2:
                        filler.extend(v_units(b + 1, lt))
                    else:
                        vdef1.append(list(v_units(b + 1, lt)))

            for it in range(TI):
                i0 = it * 512
                if last and it >= 2:
                    # v work for this i-tile must be emitted before its
                    # y_pairs reference the vaug stripes
                    for u in vdef1[it - 2]:
                        u()
                    vdef1[it - 2] = []
                # single stream per head with a TWO-deep stagger: each
                # exp gets two pair-times before its y_pair consumes it,
                # so steady state needs only ~1 filler unit per pair
                for h in range(HPC):
                    s_pair, y_pair, norm, npair = make_stream(b, it, h)
                    s_pair(0)
                    s_pair(1)
                    for p in range(2, npair):
                        s_pair(p)
                        pump(3)
                        y_pair(p - 2)
                    pump(2)
                    y_pair(npair - 2)
                    pump(2)
                    y_pair(npair - 1)
                    norm()

                # ---- phase C for this i-column: deferred via filler ----
                filler.extend(c_units(b, i0, yts[b]))

        pump(len(filler))

    nc.compile()
    return nc


def make_in_maps(x, Wqkv, bqkv, Wproj, bproj):
    Bx, Tx, Cx = x.shape
    bf = ml_dtypes.bfloat16
    BL = Bx // DPC
    R = BL * Tx
    # per-dp-group transposed activations
    xTh = []
    for dp in range(DPC):
        xg = x[dp * BL:(dp + 1) * BL].reshape(R, Cx)
        xTh.append(np.ascontiguousarray(
            xg.T.astype(np.float32)).astype(bf))
    ident_h = np.eye(128, dtype=np.float32)
    ones64_h = np.ones((65, 64), np.float32)
    f8 = ml_dtypes.float8_e4m3
    xT8h = [np.ascontiguousarray(
        x[dp * BL:(dp + 1) * BL].reshape(R, Cx).T.astype(np.float32))
        .astype(f8) for dp in range(DPC)]
    S = BL * HPC * (Tx // 128)
    vones_h = np.ones((128, S), np.float32)
    m128 = np.triu(np.ones((128, 128), np.float32))
    m256 = np.zeros((128, 256), np.float32)
    for r in range(128):
        m256[r, 128 + r:] = 1.0
    masks_h = np.concatenate([m128, m256], axis=1).astype(bf)
    assert not np.any(bqkv) and not np.any(bproj), \
        "nonzero biases unsupported in this build"
    in_maps = []
    for i in range(NCORES):
        tp, dp = i % TPC, i // TPC
        cs = slice(tp * FPC, (tp + 1) * FPC)
        # q stored as 32*Wq/8 = 4*Wq, k as 32*Wk (descale 1/32 on copy out)
        wq = Wqkv[:, 0 * C:1 * C][:, cs] * (SCALE * 32.0)
        wk = Wqkv[:, 1 * C:2 * C][:, cs] * 32.0
        wvs = Wqkv[:, 2 * C:3 * C][:, cs]
        # ci order: q_hg0, q_hg1, k_hg0, k_hg1 (fp8); v_hg0, v_hg1 (bf16)
        w8_s = np.ascontiguousarray(np.concatenate(
            [wq[:, 0:128], wq[:, 128:256],
             wk[:, 0:128], wk[:, 128:256]], axis=1)
            .astype(np.float32)).astype(f8)
        wv_s = np.ascontiguousarray(
            wvs.astype(np.float32)).astype(bf)
        wp_s = np.ascontiguousarray(Wproj[cs, :].astype(np.float32)).astype(bf)
        in_maps.append({
            "xT": xTh[dp],
            "xT8": xT8h[dp],
            "w8": w8_s,
            "wv": wv_s,
            "wp": wp_s,
            "ident": ident_h.astype(bf),
            "ones64": ones64_h.astype(bf),
            "vones": vones_h.astype(bf),
            "masks": masks_h,
        })
    return in_maps


def kernel(x, Wqkv, bqkv, Wproj, bproj, trace=False):
    global LAST_RESULT
    x = np.asarray(x, dtype=np.float32)
    Wqkv = np.asarray(Wqkv, dtype=np.float32)
    bqkv = np.asarray(bqkv, dtype=np.float32)
    Wproj = np.asarray(Wproj, dtype=np.float32)
    bproj = np.asarray(bproj, dtype=np.float32)
    Bx, Tx, Cx = x.shape
    assert Cx == C

    key = (Bx, Tx)
    if key not in _CACHE:
        _CACHE[key] = build_program(Bx, Tx)
    nc = _CACHE[key]

    in_maps = make_in_maps(x, Wqkv, bqkv, Wproj, bproj)
    res = run_bass_kernel_spmd(nc, in_maps, list(range(NCORES)), trace=trace)
    LAST_RESULT = res
    BL = Bx // DPC
    out = np.empty((Bx, Tx, Cx), dtype=np.float32)
    for dp in range(DPC):
        acc = np.zeros((C, BL * Tx), dtype=np.float32)
        for tp in range(TPC):
            acc += res.results[dp * TPC + tp]["outT"].astype(np.float32)
        out[dp * BL:(dp + 1) * BL] = \
            np.ascontiguousarray(acc.T).reshape(BL, Tx, Cx)
    return out


# revision 38
# speedup vs baseline: 1.1158x; 1.1158x over previous
"""Causal self-attention (B=4, T=2048, C=1024, H=16) on 8 trn2 NeuronCores.

Sharding: 4-way tensor parallel over heads x 2-way data parallel over batch.
Core i handles head group tp = i % 4 (4 heads) for batches [2*dp, 2*dp+1],
dp = i // 4.  Each core:
  - Wqkv columns for its 4 heads ([1024, 768] bf16, q-part pre-scaled 1/8)
  - Wproj rows for its heads ([256, 1024] bf16)
  - x for its 2 batches, transposed to [C, 2T] bf16 on host
Each core computes a partial projection [C, 2T] bf16; the host sums the 4
TP partials per batch pair in fp32 and un-transposes.

All PE operands are bf16 (f32r streams ~1.5-2x slower on HW and fp32
weight loads are 2x slower).  On-core dataflow:
  A) QKV^T per 512-token l-tile: 6 single-bank accumulation passes
     (q/k/v x 2 head-groups); q/k copied to SBUF bf16; v staged bf16 and
     PE-transposed (2 heads per transpose) into V-natural layout vaug
     with a trailing ones row (row sums l for free in PV).
  B) attention per (b, h, 512-wide i-tile): S^T j-tile pairs share a
     2-bank PSUM tile and a common query window (both halves start at the
     even tile's clip offset), ONE exp per pair (bf16 out), causal masking
     via two small bf16 mask multiplies on DVE, Y^T accumulated per j-tile.
     Softmax normalization: ACT copies l to SBUF, DVE fast reciprocal,
     gpsimd partition_broadcast to 64 lanes, one DVE multiply.
  C) output projection per i-tile (2-pass K=256 accumulation, bf16 copy
     out on DVE -> DMA).
  QKV work for batch b+1 and projection work are emitted as fine-grained
  FILLER between attention pairs so the tensor queue never drains.
"""

import numpy as np
from contextlib import ExitStack

import ml_dtypes

import concourse.bacc as bacc
import concourse.bass as bass
import concourse.mybir as mybir
import concourse.tile as tile
from concourse.bass_utils import run_bass_kernel_spmd

NCORES = 8
TPC = 4                # tensor-parallel cores (head groups)
DPC = 2                # data-parallel groups
C = 1024
H = 16
D = 64                 # head dim
HPC = H // TPC         # heads per core = 4
HG = HPC // 2          # head groups of 2 per core = 2
FPC = HPC * D          # features per core = 256
KC = C // 128          # contraction chunks = 8
SCALE = 1.0 / 8.0      # 1/sqrt(D)

F32 = mybir.dt.float32
BF16 = mybir.dt.bfloat16
FP8 = mybir.dt.float8e4
AF = mybir.ActivationFunctionType

_CACHE = {}
LAST_RESULT = None


def build_program(B, T):
    BL = B // DPC          # local batches = 2
    R = BL * T             # local tokens = 4096
    TJ = T // 128          # 128-wide j (key) tiles per sequence = 16
    TI = T // 512          # 512-wide i (query) tiles per sequence = 4
    SB = HPC * TJ          # vaug stripes per batch = 64
    assert T % 512 == 0

    nc = bacc.Bacc("TRN2", target_bir_lowering=False, debug=False,
                   num_devices=NCORES)
    xT = nc.dram_tensor("xT", [C, R], BF16, kind="ExternalInput").ap()
    xT8 = nc.dram_tensor("xT8", [C, R], FP8, kind="ExternalInput").ap()
    w8 = nc.dram_tensor("w8", [C, 2 * FPC], FP8,
                        kind="ExternalInput").ap()
    wv = nc.dram_tensor("wv", [C, FPC], BF16,
                        kind="ExternalInput").ap()
    wp = nc.dram_tensor("wp", [FPC, C], BF16, kind="ExternalInput").ap()
    ident = nc.dram_tensor("ident", [128, 128], BF16,
                           kind="ExternalInput").ap()
    ones64 = nc.dram_tensor("ones64", [65, 64], BF16,
                            kind="ExternalInput").ap()
    vones = nc.dram_tensor("vones", [128, BL * SB], BF16,
                           kind="ExternalInput").ap()
    # masks[:, 0:128]  = tril (keep col >= row)
    # masks[:, 128:384] = keep col-128 >= row (odd half of a pair window)
    masks = nc.dram_tensor("masks", [128, 384], BF16,
                           kind="ExternalInput").ap()
    outT = nc.dram_tensor("outT", [C, R], BF16, kind="ExternalOutput").ap()

    with tile.TileContext(nc) as tc, ExitStack() as ctx:
        const = ctx.enter_context(tc.tile_pool(name="const", bufs=1))
        big = ctx.enter_context(tc.tile_pool(name="big", bufs=1))
        xpool = ctx.enter_context(tc.tile_pool(name="xpool", bufs=32))
        xpool8 = ctx.enter_context(tc.tile_pool(name="xpool8", bufs=16))
        vspool = ctx.enter_context(tc.tile_pool(name="vspool", bufs=2))
        ptpool = ctx.enter_context(tc.tile_pool(name="ptpool", bufs=6))
        lpool = ctx.enter_context(tc.tile_pool(name="lpool", bufs=2))
        rpool = ctx.enter_context(tc.tile_pool(name="rpool", bufs=2))
        bcpool = ctx.enter_context(tc.tile_pool(name="bcpool", bufs=2))
        ystpool = ctx.enter_context(tc.tile_pool(name="ystpool", bufs=2))
        opool = ctx.enter_context(tc.tile_pool(name="opool", bufs=3))
        psA = ctx.enter_context(tc.tile_pool(name="psA", bufs=2, space="PSUM"))
        psS = ctx.enter_context(tc.tile_pool(name="psS", bufs=2, space="PSUM"))
        psY = ctx.enter_context(tc.tile_pool(name="psY", bufs=2, space="PSUM"))

        # ---- constants (ordered so phase A can start early) ----
        w8_sb = const.tile([128, KC // 2, 2, 2 * FPC], FP8)
        nc.sync.dma_start(out=w8_sb,
                          in_=w8.rearrange("(kp two p) c -> p kp two c",
                                           two=2, p=128))
        mask_sb = const.tile([128, 384], BF16)
        nc.sync.dma_start(out=mask_sb, in_=masks)
        wv_sb = const.tile([128, KC, FPC], BF16)
        nc.sync.dma_start(out=wv_sb,
                          in_=wv.rearrange("(kc p) c -> p kc c", p=128))
        identcol = const.tile([128, 128], BF16)
        nc.sync.dma_start(out=identcol, in_=ident)
        ones_sb = const.tile([65, 64], BF16)
        nc.sync.dma_start(out=ones_sb, in_=ones64)
        wp_sb = const.tile([128, 2, C], BF16)
        nc.sync.dma_start(out=wp_sb,
                          in_=wp.rearrange("(kc p) c -> p kc c", p=128))

        # per-batch persistent tiles
        qkts, yts, vaugs = [], [], []
        for b in range(BL):
            qkts.append(big.tile([128, HG, 2, T], BF16, name=f"qkt{b}",
                                 tag=f"qkt{b}"))
            yts.append(big.tile([128, 2, T], BF16, name=f"yt{b}",
                                tag=f"yt{b}"))
            v = big.tile([128, D + 1, SB], BF16, name=f"va{b}", tag=f"va{b}")
            nc.sync.dma_start(out=v[:, D, :],
                              in_=vones[:, b * SB:(b + 1) * SB])
            vaugs.append(v)

        xT3 = xT.rearrange("(kc p) r -> p kc r", p=128)
        xT83 = xT8.rearrange("(kp two p) r -> p kp two r", two=2, p=128)
        outTr = outT.rearrange("(cc p) r -> cc p r", p=128)

        # ---------- phase A unit generators (QKV projection) ----------
        xts_store = {}

        def ci_units(b, lt, ci):
            """Accumulation matmuls + 1 epilogue unit for one 128-col
            weight slice of l-tile (b, lt).  q/k slices (ci<4) run as
            fp8 DoubleRow over k-chunk pairs; v slices run bf16."""
            qkt, vaug = qkts[b], vaugs[b]
            l0 = lt * 512
            store = xts_store[(b, lt)]
            ps_box = []

            if ci < 4:
                def a_mm8(kp, ci=ci, ps_box=ps_box, store=store):
                    if not ps_box:
                        ps_box.append(
                            psA.tile([128, 512], F32, tag="a", name="psa"))
                    nc.tensor.matmul(
                        ps_box[0][:, :],
                        lhsT=w8_sb[:, kp, :, ci * 128:(ci + 1) * 128],
                        rhs=store["f8"][kp][:, :, :],
                        start=(kp == 0), stop=(kp == KC // 2 - 1),
                        perf_mode=mybir.MatmulPerfMode.DoubleRow,
                    )

                for kp in range(KC // 2):
                    yield lambda kp=kp, f=a_mm8: f(kp)

                # q stored as 32*Wq/8, k as 32*Wk: descale on copy out
                def a_copy(ci=ci, ps_box=ps_box, l0=l0, qkt=qkt):
                    qk, hg = ci // 2, ci % 2
                    nc.scalar.activation(
                        qkt[:, hg, qk, l0:l0 + 512], ps_box[0][:, :],
                        AF.Copy, scale=(1.0 / 32.0))

                yield a_copy
                return

            def a_mm(k, ci=ci, ps_box=ps_box, store=store):
                if not ps_box:
                    ps_box.append(
                        psA.tile([128, 512], F32, tag="a", name="psa"))
                nc.tensor.matmul(
                    ps_box[0][:, :],
                    lhsT=wv_sb[:, k, (ci - 4) * 128:(ci - 3) * 128],
                    rhs=store["bf"][k][:, :],
                    start=(k == 0), stop=(k == KC - 1),
                )

            for k in range(KC):
                yield lambda k=k, f=a_mm: f(k)

            if False:
                yield None
            else:
                # v head-group hg = ci-4: stage, transpose, store
                def v_stage(ci=ci, ps_box=ps_box, lt=lt, vaug=vaug):
                    hg = ci - 4
                    vstage = vspool.tile([128, 512], BF16, tag="vs",
                                         name="vstage")
                    nc.scalar.activation(vstage[:, :],
                                         ps_box[0][:, :], AF.Copy)
                    ps_t = psA.tile([128, 512], BF16, tag="a",
                                    name="pst")
                    for jb in range(4):
                        nc.tensor.matmul(
                            ps_t[:, jb * 128:(jb + 1) * 128],
                            lhsT=vstage[:, jb * 128:(jb + 1) * 128],
                            rhs=identcol[:, :],
                            is_transpose=True,
                            start=(jb == 0), stop=(jb == 3),
                            skip_group_check=True,
                        )
                    # stripes: head h=2*hg+hh at (2*hg+hh)*TJ + lt*4
                    va_r = vaug[:, 0:D, :].rearrange(
                        "p d (h t j) -> p d h t j",
                        h=HPC, t=TI, j=4)
                    nc.vector.tensor_copy(
                        va_r[:, :, 2 * hg:2 * hg + 2, lt, :],
                        ps_t.rearrange("p (jb hh d) -> p d hh jb",
                                       jb=4, hh=2))

                yield v_stage

        def v_units(b, lt):
            r0 = b * T + lt * 512
            store = xts_store[(b, lt)]

            def dma_unit(k, r0=r0, store=store):
                xt = xpool.tile([128, 512], BF16, tag="xt",
                                name=f"xtv{b}_{k}")
                nc.sync.dma_start(out=xt,
                                  in_=xT3[:, k, r0:r0 + 512])
                store["bf"].append(xt)

            for k in range(KC):
                yield lambda k=k, f=dma_unit: f(k)
            for ci in (4, 5):
                yield from ci_units(b, lt, ci)

        def a_units(b, defer_v=False):
            """Yield emission closures for batch b's QKV projection."""
            for lt in range(TI):
                r0 = b * T + lt * 512
                store = xts_store.setdefault((b, lt),
                                             {"bf": [], "f8": []})

                def dma8_unit(kp, r0=r0, store=store):
                    xt = xpool8.tile([128, 2, 512], FP8, tag="x8",
                                     name=f"x8{b}_{kp}")
                    nc.sync.dma_start(out=xt,
                                      in_=xT83[:, kp, :, r0:r0 + 512])
                    store["f8"].append(xt)

                for kp in range(KC // 2):
                    yield lambda kp=kp, f=dma8_unit: f(kp)

                for ci in range(4):
                    yield from ci_units(b, lt, ci)
                if not defer_v:
                    yield from v_units(b, lt)

        filler = []

        def pump(n):
            for _ in range(n):
                if not filler:
                    return
                filler.pop(0)()

        # ---------- phase C unit generator (output projection) ----------
        def c_units(b, i0, yt):
            w0 = b * T + i0
            for ct in range(KC):
                box = []

                def c_mm(kc, ct=ct, box=box, yt=yt, i0=i0):
                    if not box:
                        box.append(psA.tile([128, 512], F32, tag="a",
                                            name="pso"))
                    nc.tensor.matmul(
                        box[0][:, :],
                        lhsT=wp_sb[:, kc, ct * 128:(ct + 1) * 128],
                        rhs=yt[:, kc, i0:i0 + 512],
                        start=(kc == 0), stop=(kc == 1),
                    )

                for kc in range(2):
                    yield lambda kc=kc, f=c_mm: f(kc)

                def c_out(ct=ct, box=box, w0=w0):
                    ost = opool.tile([128, 512], BF16, tag="o", name="ost")
                    nc.vector.tensor_copy(ost[:, :], box[0][:, :])
                    nc.sync.dma_start(
                        out=outTr[ct, :, w0:w0 + 512],
                        in_=ost[:, :],
                    )

                yield c_out

        # ---------- phase B stream factory ----------
        def make_stream(b, it, h):
            qkt, yt, vaug = qkts[b], yts[b], vaugs[b]
            i0 = it * 512
            njt = (i0 + 512) // 128
            npair = njt // 2
            hg, hh = h // 2, h % 2
            p0 = 64 * hh
            ps_y = psY.tile([D + 1, 512], F32, tag="y", name="psy")
            pts = [None] * npair

            def s_pair(p):
                # both halves share the even tile's query window; the odd
                # strip is laid contiguously at [w, 2w) so one exp covers
                # exactly the written region
                off = max(0, 2 * p * 128 - i0)
                w = 512 - off
                straddle = (2 * p + 1) * 128 > i0
                ps_s = psS.tile([128, 1024], F32, tag="s", name="pss")
                pt = ptpool.tile([128, 1024], BF16, tag="pt", name="pt")
                for half in range(2):
                    j0 = (2 * p + half) * 128
                    nc.tensor.matmul(
                        ps_s[:, half * w:half * w + w],
                        lhsT=qkt[p0:p0 + 64, hg, 1, j0:j0 + 128],
                        rhs=qkt[p0:p0 + 64, hg, 0, i0 + off:i0 + 512],
                        start=True, stop=True,
                    )
                nc.scalar.activation(pt[:, 0:2 * w], ps_s[:, 0:2 * w],
                                     AF.Exp)
                if straddle:
                    nc.vector.tensor_tensor(
                        out=pt[:, 0:128], in0=pt[:, 0:128],
                        in1=mask_sb[:, 0:128],
                        op=mybir.AluOpType.mult)
                    nc.vector.tensor_tensor(
                        out=pt[:, w:w + 256],
                        in0=pt[:, w:w + 256],
                        in1=mask_sb[:, 128:384],
                        op=mybir.AluOpType.mult)
                pts[p] = pt

            def y_pair(p):
                pt = pts[p]
                off = max(0, 2 * p * 128 - i0)
                w = 512 - off
                for half in range(2):
                    jj = 2 * p + half
                    nc.tensor.matmul(
                        ps_y[:, off:512],
                        lhsT=vaug[:, :, h * TJ + jj],
                        rhs=pt[:, half * w:half * w + w],
                        start=(jj == 0), stop=(jj == njt - 1),
                    )

            def norm():
                # l -> SBUF f32r (DVE), PE ones-broadcast to 64 lanes,
                # copy out (ACT), reciprocal + multiply (DVE)
                lsb = lpool.tile([D + 1, 512], BF16, tag="l", name="lsb")
                with nc.allow_low_precision(reason="l to bf16 for bcast"):
                    nc.vector.tensor_copy(lsb[D:D + 1, :], ps_y[D:D + 1, :])
                ps_b = psS.tile([64, 512], F32, tag="s", name="psb")
                nc.tensor.matmul(ps_b[:, :], lhsT=ones_sb[64:65, :],
                                 rhs=lsb[D:D + 1, :],
                                 start=True, stop=True)
                bcl = rpool.tile([64, 512], F32, tag="r", name="bcl")
                nc.vector.tensor_copy(bcl[:, :], ps_b[:, :])
                bc = bcpool.tile([64, 512], F32, tag="bc", name="bc")
                nc.vector.reciprocal_approx_fast(out=bc[:, :], in_=bcl[:, :])
                if hh == 0:
                    nc.vector.tensor_mul(yt[0:64, hg, i0:i0 + 512],
                                         ps_y[0:D, :], bc[:, :])
                else:
                    yst = ystpool.tile([64, 512], BF16, tag="yst",
                                       name="yst")
                    nc.vector.tensor_mul(yst[:, :], ps_y[0:D, :], bc[:, :])
                    nc.sync.dma_start(out=yt[64:128, hg, i0:i0 + 512],
                                      in_=yst[:, :])

            return s_pair, y_pair, norm, npair

        # ---------- prologue: batch 0 q/k straight through, v deferred ----
        for u in a_units(0, defer_v=True):
            u()
        for u in v_units(0, 0):
            u()
        for lt in range(1, TI):
            filler.extend(v_units(0, lt))

        # ---------- main loop ----------
        vdef1 = []     # batch-1 v work deferred into batch 1's attention
        for b in range(BL):
            last = (b + 1 == BL)
            if not last:
                filler.extend(a_units(b + 1, defer_v=True))
                for lt in range(TI):
                    if lt < 2:
                        filler.extend(v_units(b + 1, lt))
                    else:
                        vdef1.append(list(v_units(b + 1, lt)))

            for it in range(TI):
                i0 = it * 512
                if last and it >= 2:
                    # v work for this i-tile must be emitted before its
                    # y_pairs reference the vaug stripes
                    for u in vdef1[it - 2]:
                        u()
                    vdef1[it - 2] = []
                # single stream per head with a TWO-deep stagger: each
                # exp gets two pair-times before its y_pair consumes it,
                # so steady state needs only ~1 filler unit per pair
                for h in range(HPC):
                    s_pair, y_pair, norm, npair = make_stream(b, it, h)
                    s_pair(0)
                    s_pair(1)
                    for p in range(2, npair):
                        s_pair(p)
                        pump(3)
                        y_pair(p - 2)
                    pump(2)
                    y_pair(npair - 2)
                    pump(2)
                    y_pair(npair - 1)
                    norm()

                # ---- phase C for this i-column: deferred via filler ----
                filler.extend(c_units(b, i0, yts[b]))

        pump(len(filler))

    nc.compile()
    return nc


def make_in_maps(x, Wqkv, bqkv, Wproj, bproj):
    Bx, Tx, Cx = x.shape
    bf = ml_dtypes.bfloat16
    BL = Bx // DPC
    R = BL * Tx
    # per-dp-group transposed activations
    xTh = []
    for dp in range(DPC):
        xg = x[dp * BL:(dp + 1) * BL].reshape(R, Cx)
        xTh.append(np.ascontiguousarray(
            xg.T.astype(np.float32)).astype(bf))
    ident_h = np.eye(128, dtype=np.float32)
    ones64_h = np.ones((65, 64), np.float32)
    f8 = ml_dtypes.float8_e4m3
    xT8h = [np.ascontiguousarray(
        x[dp * BL:(dp + 1) * BL].reshape(R, Cx).T.astype(np.float32))
        .astype(f8) for dp in range(DPC)]
    S = BL * HPC * (Tx // 128)
    vones_h = np.ones((128, S), np.float32)
    m128 = np.triu(np.ones((128, 128), np.float32))
    m256 = np.zeros((128, 256), np.float32)
    for r in range(128):
        m256[r, 128 + r:] = 1.0
    masks_h = np.concatenate([m128, m256], axis=1).astype(bf)
    assert not np.any(bqkv) and not np.any(bproj), \
        "nonzero biases unsupported in this build"
    in_maps = []
    for i in range(NCORES):
        tp, dp = i % TPC, i // TPC
        cs = slice(tp * FPC, (tp + 1) * FPC)
        # q stored as 32*Wq/8 = 4*Wq, k as 32*Wk (descale 1/32 on copy out)
        wq = Wqkv[:, 0 * C:1 * C][:, cs] * (SCALE * 32.0)
        wk = Wqkv[:, 1 * C:2 * C][:, cs] * 32.0
        wvs = Wqkv[:, 2 * C:3 * C][:, cs]
        # ci order: q_hg0, q_hg1, k_hg0, k_hg1 (fp8); v_hg0, v_hg1 (bf16)
        w8_s = np.ascontiguousarray(np.concatenate(
            [wq[:, 0:128], wq[:, 128:256],
             wk[:, 0:128], wk[:, 128:256]], axis=1)
            .astype(np.float32)).astype(f8)
        wv_s = np.ascontiguousarray(
            wvs.astype(np.float32)).astype(bf)
        wp_s = np.ascontiguousarray(Wproj[cs, :].astype(np.float32)).astype(bf)
        in_maps.append({
            "xT": xTh[dp],
            "xT8": xT8h[dp],
            "w8": w8_s,
            "wv": wv_s,
            "wp": wp_s,
            "ident": ident_h.astype(bf),
            "ones64": ones64_h.astype(bf),
            "vones": vones_h.astype(bf),
            "masks": masks_h,
        })
    return in_maps


def kernel(x, Wqkv, bqkv, Wproj, bproj, trace=False):
    global LAST_RESULT
    x = np.asarray(x, dtype=np.float32)
    Wqkv = np.asarray(Wqkv, dtype=np.float32)
    bqkv = np.asarray(bqkv, dtype=np.float32)
    Wproj = np.asarray(Wproj, dtype=np.float32)
    bproj = np.asarray(bproj, dtype=np.float32)
    Bx, Tx, Cx = x.shape
    assert Cx == C

    key = (Bx, Tx)
    if key not in _CACHE:
        _CACHE[key] = build_program(Bx, Tx)
    nc = _CACHE[key]

    in_maps = make_in_maps(x, Wqkv, bqkv, Wproj, bproj)
    res = run_bass_kernel_spmd(nc, in_maps, list(range(NCORES)), trace=trace)
    LAST_RESULT = res
    BL = Bx // DPC
    out = np.empty((Bx, Tx, Cx), dtype=np.float32)
    for dp in range(DPC):
        acc = np.zeros((C, BL * Tx), dtype=np.float32)
        for tp in range(TPC):
            acc += res.results[dp * TPC + tp]["outT"].astype(np.float32)
        out[dp * BL:(dp + 1) * BL] = \
            np.ascontiguousarray(acc.T).reshape(BL, Tx, Cx)
    return out
